# revision 1
# baseline (speedup 1.0000x reference)
"""Trainium2 Bass kernel: causal multi-head attention with RoPE.

Model: B=2, S=2048, D=2048, H=16 heads, head_dim=128, fp32.

Sharding (8 cores): batch (2) x head-groups (4 heads each).  Each core
computes q/k/v projections for its 4 heads, head-local attention, and a
partial output projection (row-slice of wo); the host sums the 4 partials
per batch (the tensor-parallel all-reduce done on host).

Device-side layout trick: q and k are produced directly in transposed
[head_dim, seq] layout by using the weight tile as the stationary matmul
operand.  Scores are computed transposed ([k, q]) so that:
  - the softmax denominator is a ones-vector matmul on the PE (partition
    direction sum), accumulated across k-chunks in PSUM;
  - P @ V needs no transpose (V in natural [k, head_dim] layout is the
    stationary operand, exp(scores^T) the moving one), producing the
    attention output directly in [head_dim, seq] layout;
  - that output feeds the wo matmul directly as the stationary operand.
RoPE pairs (even/odd feature columns) are made contiguous halves by
permuting wq/wk columns on the host, so the on-chip rotation is plain
half-tile elementwise ops.  Softmax is computed without max-subtraction
(scores are O(6) for this problem size/scale, exp is safe in fp32).
qT and kT spill to DRAM scratch between projection and attention phases to fit
SBUF; everything else stays resident.
"""

import math
import os
import sys

import numpy as np

for _p in ("/opt/trn_rl_repo", "/root/.axon_site/_ro/trn_rl_repo"):
    if os.path.isdir(_p) and _p not in sys.path:
        sys.path.insert(0, _p)

import concourse.bacc as bacc
import concourse.mybir as mybir
from concourse import tile
from concourse.bass_utils import run_bass_kernel_spmd

F32 = mybir.dt.float32
F32R = mybir.dt.float32r

B, S, D, H, HD = 2, 2048, 2048, 16, 128
NCORES = 8
HPC = 4          # heads per core
HGRP = NCORES // B  # head groups (4)
FPC = HPC * HD   # features per core (512)
T5 = S // 512    # number of 512-wide seq tiles
DC = D // 128    # number of 128-deep contraction chunks
SC = 1.0 / math.sqrt(HD)

# Use the PE's reduced-precision fp32 mode (1.5 cyc/row vs 2.0) when True.
# All matmul operands (and their producers) are declared float32r end-to-end,
# as the BIR verifier requires; float32r maps to np.float32 on the host.
USE_F32R = True


def _build_program(mode, f32r=USE_F32R):
    """Trace the single-core SPMD program.  mode: 'causal' | 'dense' | 'general'."""
    nc = bacc.Bacc("TRN2", target_bir_lowering=False, debug=False,
                   num_devices=NCORES)
    MDT = F32R if f32r else F32

    xT = nc.dram_tensor("xT", [D, S], MDT, kind="ExternalInput")
    wq = nc.dram_tensor("wq", [D, FPC], MDT, kind="ExternalInput")
    wk = nc.dram_tensor("wk", [D, FPC], MDT, kind="ExternalInput")
    wv = nc.dram_tensor("wv", [D, FPC], MDT, kind="ExternalInput")
    wo = nc.dram_tensor("wo", [FPC, D], MDT, kind="ExternalInput")
    cosT = nc.dram_tensor("cosT", [HD // 2, S], F32, kind="ExternalInput")
    sinT = nc.dram_tensor("sinT", [HD // 2, S], F32, kind="ExternalInput")
    ones_d = nc.dram_tensor("ones_d", [128, 1], MDT, kind="ExternalInput")
    if mode == "causal":
        m01 = nc.dram_tensor("m01", [4, 128, 512], MDT, kind="ExternalInput")
    if mode == "general":
        maskT = nc.dram_tensor("maskT", [S, S], F32, kind="ExternalInput")
    out = nc.dram_tensor("out", [S, D], F32, kind="ExternalOutput")

    qTd = nc.dram_tensor("qTd", [HPC, 128, S], MDT)  # internal scratch
    kTd = nc.dram_tensor("kTd", [HPC, 128, S], MDT)  # internal scratch

    def nk_of(q5):
        return 4 * (q5 + 1) if mode == "causal" else DC

    with tile.TileContext(nc, pool_alloc_mode='queue') as tc:
        with (
            tc.tile_pool(name="persist", bufs=1) as pp,
            tc.tile_pool(name="ktp", bufs=1) as ktpool,
            tc.tile_pool(name="qa_ps", bufs=6, space="PSUM") as gps,
        ):
            ones = pp.tile([128, 1], MDT, tag="ones", name="ones")
            nc.sync.dma_start(ones[:], ones_d[:])
            vsb = [pp.tile([128, FPC], MDT, tag=f"v{t}", name=f"v{t}")
                   for t in range(S // 128)]

            def load_xt(sb):
                tiles = {}
                def get(t5, reload=False, interleave=None):
                    if t5 not in tiles or reload:
                        tsl = slice(t5 * 512, (t5 + 1) * 512)
                        xt = [sb.tile([128, 512], MDT, tag="xt", bufs=32,
                                      name="xt") for _ in range(DC)]
                        for dc in range(DC):
                            nc.sync.dma_start(
                                xt[dc][:], xT[dc * 128:(dc + 1) * 128, tsl])
                            if interleave is not None:
                                dst, src_ = interleave[dc]
                                nc.sync.dma_start(dst[:], src_)
                        tiles[t5] = xt
                    return tiles[t5]
                return get

            # qk weight pool opens first so its DMAs prefetch during phase V
            with (
                tc.tile_pool(name="qk_w", bufs=1) as qwp,
                tc.tile_pool(name="xt_p", bufs=2) as xp,
            ):
                get_xt_shared = load_xt(xp)
                # ---- Phase V: v projection (natural [seq, feat] layout) ----
                with (
                    tc.tile_pool(name="v_w", bufs=1) as wp,
                    tc.tile_pool(name="v_sb", bufs=2) as sb,
                ):
                    ps = gps
                    get_xt = get_xt_shared
                    wv_t = [wp.tile([128, FPC], MDT, tag=f"wv{dc}",
                                    name=f"wv{dc}") for dc in range(DC)]
                    wv_pairs = [(wv_t[dc], wv[dc * 128:(dc + 1) * 128, :])
                                for dc in range(DC)]
                    xt0 = get_xt(0, interleave=wv_pairs)
                    for t5 in range(T5):
                        xt = get_xt(t5)
                        accs = [ps.tile([128, 512], F32, tag="mm", name="vps")
                                for _ in range(4)]
                        for dc in range(DC):
                            for t in range(4):
                                nc.tensor.matmul(
                                    accs[t][:],
                                    (xt[dc][:, t * 128:(t + 1) * 128]),
                                    (wv_t[dc][:]),
                                    start=(dc == 0), stop=(dc == DC - 1))
                        for t in range(4):
                            nc.scalar.copy(vsb[t5 * 4 + t][:], accs[t][:])

                # q/k weights: prefetch behind phase V's tail
                wq_t = [qwp.tile([128, FPC], MDT, tag=f"wq{dc}",
                                 name=f"wq{dc}") for dc in range(DC)]
                wk_t = [qwp.tile([128, FPC], MDT, tag=f"wk{dc}",
                                 name=f"wk{dc}") for dc in range(DC)]
                for dc in range(DC):
                    nc.sync.dma_start(wq_t[dc][:],
                                      wq[dc * 128:(dc + 1) * 128, :])
                for dc in range(DC):
                    nc.sync.dma_start(wk_t[dc][:],
                                      wk[dc * 128:(dc + 1) * 128, :])

                # ---- Phase QK: q/k projections (transposed) + RoPE ----
                with (
                    tc.tile_pool(name="qk_sb", bufs=2) as sb,
                ):
                    ps = gps
                    get_xt = get_xt_shared
                    for t5 in [3, 2, 0, 1]:
                        tsl = slice(t5 * 512, (t5 + 1) * 512)
                        xt = get_xt(t5, reload=(t5 in (0, 1)))
                        ct = sb.tile([64, 512], F32, tag="cos", bufs=2)
                        st = sb.tile([64, 512], F32, tag="sin", bufs=2)
                        nc.sync.dma_start(ct[:], cosT[:, tsl])
                        nc.sync.dma_start(st[:], sinT[:, tsl])
                        for h in range(HPC):
                            hsl = slice(h * 128, (h + 1) * 128)
                            for w_t, dstd in ((wq_t, qTd), (wk_t, kTd)):
                                acc = ps.tile([128, 512], F32, tag="mm", name="qkps")
                                for dc in range(DC):
                                    nc.tensor.matmul(
                                        acc[:], (w_t[dc][:, hsl]),
                                        (xt[dc][:]),
                                        start=(dc == 0), stop=(dc == DC - 1))
                                # RoPE: rows 0:64 = "a" (even), 64:128 = "b"
                                a, b = acc[0:64, :], acc[64:128, :]
                                m1 = sb.tile([64, 512], F32, tag="m1", bufs=3)
                                m2 = sb.tile([64, 512], F32, tag="m2", bufs=2)
                                m3 = sb.tile([64, 512], F32, tag="m3", bufs=2)
                                m4 = sb.tile([64, 512], F32, tag="m4", bufs=2)
                                nc.vector.tensor_mul(m1[:], a, ct[:])
                                nc.vector.tensor_mul(m2[:], b, st[:])
                                nc.vector.tensor_mul(m3[:], a, st[:])
                                nc.vector.tensor_mul(m4[:], b, ct[:])
                                rt = sb.tile([128, 512], MDT, tag="rt", bufs=3)
                                nc.gpsimd.tensor_sub(rt[0:64, :], m1[:], m2[:])
                                nc.gpsimd.tensor_add(rt[64:128, :], m3[:], m4[:])
                                nc.sync.dma_start(dstd[h][:, tsl], rt[:])

            # ---- Phase A: attention; Phase W: output projection ----
            with (
                tc.tile_pool(name="at_p", bufs=1) as ap,
                tc.tile_pool(name="wo_w", bufs=1) as wp,
            ):
                attnT = [ap.tile([128, S], MDT, tag=f"aT{h}", name=f"aT{h}")
                         for h in range(HPC)]
                wo_t = [[wp.tile([128, 512], MDT, tag=f"wo{h}_{o5}",
                                 name=f"wo{h}_{o5}")
                         for o5 in range(4)] for h in range(HPC)]
                with (
                    tc.tile_pool(name="a_sb", bufs=2) as sb,
                ):
                    ps = gps
                    if mode == "causal":
                        m01_t = [sb.tile([128, 512], MDT, tag=f"m01_{r}",
                                         bufs=1, name=f"m01_{r}")
                                 for r in range(4)]
                        for r in range(4):
                            nc.sync.dma_start(m01_t[r][:], m01[r])
                    for h in range(HPC):
                        kt = ktpool.tile([128, S], MDT, tag="kt", bufs=1,
                                         name="kt")
                        nc.sync.dma_start(kt[:], kTd[h][:, :])
                        for q5 in range(T5):
                            qsl = slice(q5 * 512, (q5 + 1) * 512)
                            nk = nk_of(q5)
                            qt = ktpool.tile([128, 512], MDT, tag="qt",
                                             bufs=3, name="qt")
                            nc.sync.dma_start(qt[:], qTd[h][:, qsl])
                            aps = ps.tile([128, 512], F32, tag="acc", bufs=2,
                                          name="aps")
                            dps = ps.tile([1, 512], F32, tag="acc", bufs=2,
                                          name="dps")
                            for kc in range(nk):
                                sps = ps.tile([128, 512], F32, tag="mm",
                                              bufs=6, name="sps")
                                nc.tensor.matmul(
                                    sps[:],
                                    (kt[:, kc * 128:(kc + 1) * 128]),
                                    (qt[:]),
                                    start=True, stop=True)
                                e = sb.tile([128, 512], MDT, tag="e", bufs=18)
                                r = kc - (nk - 4)
                                if mode == "causal" and r >= 0:
                                    nc.scalar.activation(
                                        e[:], sps[:],
                                        mybir.ActivationFunctionType.Exp,
                                        scale=SC)
                                    nc.vector.tensor_mul(e[:], e[:],
                                                         m01_t[r][:])
                                elif mode == "general":
                                    g = sb.tile([128, 512], F32, tag="gm",
                                                bufs=3)
                                    nc.sync.dma_start(
                                        g[:],
                                        maskT[kc * 128:(kc + 1) * 128, qsl])
                                    sm = sb.tile([128, 512], F32, tag="sm",
                                                 bufs=3)
                                    nc.vector.tensor_add(sm[:], sps[:], g[:])
                                    nc.scalar.activation(
                                        e[:], sm[:],
                                        mybir.ActivationFunctionType.Exp,
                                        scale=SC)
                                else:
                                    nc.scalar.activation(
                                        e[:], sps[:],
                                        mybir.ActivationFunctionType.Exp,
                                        scale=SC)
                                nc.tensor.matmul(
                                    dps[:], (ones[:]), (e[:]),
                                    start=(kc == 0), stop=(kc == nk - 1))
                                nc.tensor.matmul(
                                    aps[:],
                                    (vsb[kc][:, h * 128:(h + 1) * 128]),
                                    (e[:]),
                                    start=(kc == 0), stop=(kc == nk - 1))
                            r1 = sb.tile([1, 512], F32, tag="r1", bufs=3)
                            nc.vector.reciprocal(r1[:], dps[:])
                            rb = sb.tile([128, 512], F32, tag="rb", bufs=3)
                            nc.gpsimd.partition_broadcast(rb[:], r1[:])
                            nc.vector.tensor_mul(attnT[h][:, qsl], aps[:],
                                                 rb[:])
                        if h == 0:
                            for hh in range(HPC):
                                for o5 in range(4):
                                    nc.sync.dma_start(
                                        wo_t[hh][o5][:],
                                        wo[hh * 128:(hh + 1) * 128,
                                           o5 * 512:(o5 + 1) * 512])

                # ---- Phase W ----
                with (
                    tc.tile_pool(name="w_sb", bufs=2) as sb,
                ):
                    ps = gps
                    for tt in range(S // 128):
                        for o5 in range(4):
                            acc = ps.tile([128, 512], F32, tag="mm", name="ops")
                            for h in range(HPC):
                                nc.tensor.matmul(
                                    acc[:],
                                    (attnT[h][:, tt * 128:(tt + 1) * 128]),
                                    (wo_t[h][o5][:]),
                                    start=(h == 0), stop=(h == HPC - 1))
                            ot = sb.tile([128, 512], F32, tag="ot", bufs=6)
                            nc.scalar.copy(ot[:], acc[:])
                            nc.sync.dma_start(
                                out[tt * 128:(tt + 1) * 128,
                                    o5 * 512:(o5 + 1) * 512],
                                ot[:])

    nc.finalize()
    return nc


_PROGRAMS = {}


def _get_program(mode, f32r=None):
    if f32r is None:
        f32r = USE_F32R
    key = (mode, f32r)
    if key not in _PROGRAMS:
        _PROGRAMS[key] = _build_program(mode, f32r)
    return _PROGRAMS[key]


def _rope_perm():
    p = np.empty(HD, np.int64)
    p[: HD // 2] = np.arange(0, HD, 2)
    p[HD // 2:] = np.arange(1, HD, 2)
    return p


def _detect_mode(mask2):
    if not np.any(mask2):
        return "dense"
    iu = np.triu_indices(S, 1)
    il = np.tril_indices(S, 0)
    if not np.any(mask2[il]) and np.all(mask2[iu] <= -1.0e4):
        return "causal"
    return "general"


def _prepare_inputs(x, wq, wk, wv, wo, cos, sin, mask, start_p, seq_l):
    x = np.asarray(x, np.float32)
    wq = np.asarray(wq, np.float32)
    wk = np.asarray(wk, np.float32)
    wv = np.asarray(wv, np.float32)
    wo = np.asarray(wo, np.float32)
    cos = np.asarray(cos, np.float32)
    sin = np.asarray(sin, np.float32)
    mask2 = np.asarray(mask, np.float32).reshape(S, S)
    sp = int(np.asarray(start_p))
    sl = int(np.asarray(seq_l))
    assert sl == S, f"kernel hardcodes seq_l == {S}, got {sl}"

    mode = _detect_mode(mask2)

    c = np.ascontiguousarray(cos[sp:sp + sl].T)  # [64, S]
    s = np.ascontiguousarray(sin[sp:sp + sl].T)

    perm = _rope_perm()
    in_maps = []
    shared = {"cosT": c, "sinT": s,
              "ones_d": np.ones((128, 1), np.float32)}
    if mode == "causal":
        i = np.arange(128)[:, None]
        j = np.arange(512)[None, :]
        m01 = np.empty((4, 128, 512), np.float32)
        for r in range(4):
            m01[r] = (j >= i + 128 * r).astype(np.float32)
        shared["m01"] = m01
    if mode == "general":
        shared["maskT"] = np.ascontiguousarray(mask2.T * math.sqrt(HD))

    xTs = [np.ascontiguousarray(x[b].T) for b in range(B)]
    for core in range(NCORES):
        b = core // HGRP
        g = core % HGRP
        hs = g * HPC  # first global head of this core
        cols = []
        for h in range(HPC):
            base = (hs + h) * HD
            cols.append(base + perm)
        cols = np.concatenate(cols)
        csl = slice(hs * HD, hs * HD + FPC)
        in_maps.append({
            "xT": xTs[b],
            "wq": np.ascontiguousarray(wq[:, cols]),
            "wk": np.ascontiguousarray(wk[:, cols]),
            "wv": np.ascontiguousarray(wv[:, csl]),
            "wo": np.ascontiguousarray(wo[csl, :]),
            **shared,
        })
    return mode, in_maps


def run(inputs, trace=False):
    mode, in_maps = _prepare_inputs(**inputs)
    nc = _get_program(mode)
    res = run_bass_kernel_spmd(nc, in_maps, list(range(NCORES)), trace=trace)
    out = np.empty((B, S, D), np.float32)
    for b in range(B):
        acc = res.results[b * HGRP]["out"].astype(np.float32)
        for g in range(1, HGRP):
            acc = acc + res.results[b * HGRP + g]["out"]
        out[b] = acc
    return out, res


def kernel(**inputs):
    out, _ = run(inputs, trace=False)
    return out



# revision 63
# speedup vs baseline: 1.2287x; 1.2287x over previous
"""Trainium2 Bass kernel: causal multi-head attention with RoPE.

Model: B=2, S=2048, D=2048, H=16 heads, head_dim=128, fp32 in/out.

Sharding (8 cores): batch (2) x head-groups (4 heads each).  Each core
computes q/k/v projections for its 4 heads, head-local attention, and a
partial output projection (row-slice of wo); the host sums the 4 partials
per batch (the tensor-parallel all-reduce done on host).

Design notes (vs the f32r baseline, ~361us -> ~294us):
  - All matmul operands are bf16 (PSUM accumulation stays fp32); matmul
    throughput is identical to f32r (1 cyc/row at N>=256) but SBUF/DMA
    traffic halves, so qT/kT stay SBUF-resident (no DRAM spill) and the
    x tiles are loaded once (V and QK merged per seq-tile stage).
  - Fully software-pipelined single pass: attention stage q5 interleaves
    between the QK matmul blocks of stage q5+1 (its qT/kT/vsb deps are a
    full stage old, so the in-order PE queue never stalls on them), and
    output-projection (W) acc groups from a backlog fill the exp-latency
    bubbles inside the attention units one stage later again.
  - One shared PSUM pool for everything: score pair-groups [128,1024]
    double as V-projection acc pairs; W accs share a tag with the QK
    projection accs (which are staged out of PSUM by two fast ACT/DVE
    copies so their banks recycle in ~0.6us, with RoPE then running in
    bf16 2x off the staged tiles).
  - Softmax denominators: exp chunks pair/quad-reduce on DVE (bf16, 2x),
    quads accumulate into an fp32 SBUF tile on GpSimd, and one gpsimd
    partition_all_reduce yields broadcast column sums -- no ones-matmuls
    (saves ~160 N=512 PE matmuls/core) and no PSUM bank for them.
  - Causal diagonal chunks compute only the valid q-column range (moving
    width 512-128r), trimming ~15%% of attention PE+ACT work; two diag
    chunks share each 2-bank score tile; the last unit uses a
    split-column normalize so the final W groups start sooner.
  - exp is batched 2 score-chunks per activation ([128,1024] reads
    spanning 2 PSUM banks) to amortize the ACT access bubble.
  - DMAs are batched multi-chunk transfers (descriptor generation is
    serial at ~650ns/DMA), ramped fine->coarse at startup so the first V
    matmuls start at ~2.5us.

Scores are computed transposed ([k, q]) as in the baseline so softmax
denominators reduce over partitions and P@V needs no transpose; exp is
computed without max-subtraction (scores are O(6), safe in fp32).

Known-good cost-model time: 293773 ns; HW-validated rel err ~4e-3.
"""
import math
import os
import sys

import numpy as np

for _p in ("/opt/trn_rl_repo", "/root/.axon_site/_ro/trn_rl_repo"):
    if os.path.isdir(_p) and _p not in sys.path:
        sys.path.insert(0, _p)

import ml_dtypes

import concourse.bacc as bacc
import concourse.mybir as mybir
from concourse import bass_isa, tile
from concourse.bass_utils import run_bass_kernel_spmd

F32 = mybir.dt.float32
F32R = mybir.dt.float32r
BF16 = mybir.dt.bfloat16
BF = ml_dtypes.bfloat16

B, S, D, H, HD = 2, 2048, 2048, 16, 128
NCORES = 8
HPC = 4          # heads per core
HGRP = NCORES // B  # head groups (4)
FPC = HPC * HD   # features per core (512)
T5 = S // 512    # number of 512-wide seq tiles
DC = D // 128    # number of 128-deep contraction chunks
NT = S // 128    # number of 128-row seq tiles (16)
SC = 1.0 / math.sqrt(HD)


def _build_program(mode):
    """Trace the single-core SPMD program.  mode: 'causal' | 'dense' | 'general'."""
    nc = bacc.Bacc("TRN2", target_bir_lowering=False, debug=False,
                   num_devices=NCORES)

    xT = nc.dram_tensor("xT", [D, S], BF16, kind="ExternalInput")
    wq = nc.dram_tensor("wq", [D, FPC], BF16, kind="ExternalInput")
    wk = nc.dram_tensor("wk", [D, FPC], BF16, kind="ExternalInput")
    wv = nc.dram_tensor("wv", [D, FPC], BF16, kind="ExternalInput")
    wo = nc.dram_tensor("wo", [FPC, D], BF16, kind="ExternalInput")
    cs = nc.dram_tensor("cs", [64, 2 * S], BF16, kind="ExternalInput")
    if mode == "causal":
        m01 = nc.dram_tensor("m01", [128, 128], BF16, kind="ExternalInput")
    if mode == "general":
        maskT = nc.dram_tensor("maskT", [S, S], F32, kind="ExternalInput")
    out = nc.dram_tensor("out", [S, D], BF16, kind="ExternalOutput")

    def nk_of(q5):
        return 4 * (q5 + 1) if mode == "causal" else NT

    # general mode spends 8KB/part on mask staging tiles; shrink elsewhere
    EGB = 4 if mode != "general" else 2
    E1B = 4 if mode != "general" else 2
    OTB = 3 if mode != "general" else 2

    with tile.TileContext(nc, pool_alloc_mode='queue') as tc:
        with (
            tc.tile_pool(name="persist", bufs=1) as pp,
            tc.tile_pool(name="wo_w", bufs=1) as wop,
            tc.tile_pool(name="a_sb", bufs=2) as asb,
            tc.tile_pool(name="g_ps", bufs=1, space="PSUM") as ps,
        ):
            vsb = [pp.tile([128, FPC], BF16, tag=f"v{t}", name=f"v{t}")
                   for t in range(NT)]
            qT = [pp.tile([128, S], BF16, tag=f"qT{h}", name=f"qT{h}")
                  for h in range(HPC)]
            kT = [pp.tile([128, S], BF16, tag=f"kT{h}", name=f"kT{h}")
                  for h in range(HPC)]
            # attnT tiles are per (head, q5) and rotate through an
            # 8-slot tag: a tile lives from its normalize write until the
            # W matmuls one stage later consume it
            attnT = {}
            cs_t = pp.tile([64, 2 * S], BF16, tag="cs", name="cs_t")
            if mode == "causal":
                m01t = pp.tile([128, 128], BF16, tag="m01", name="m01t")
            wo5 = wop.tile([128, 16 * 512], BF16, tag="wo5", name="wo5")
            # wo5 column layout: h-major then o5: [h*4+o5] -> 512-wide slice
            wo_t = [[wo5[:, (h * 4 + o5) * 512:(h * 4 + o5 + 1) * 512]
                     for o5 in range(4)] for h in range(HPC)]

            # ---- W (output projection) machinery: backlog + stepper -----
            # W work interleaves into later attention stages so the PE
            # queue always has ready matmuls behind the normalize tails.
            ncopy = 0
            w_backlog = []
            w_ot = {}

            def w_push(q5):
                for tt in range(q5 * 4, (q5 + 1) * 4):
                    for o5 in range(4):
                        w_backlog.append((tt, o5))

            def w_step(n=1, engs=(None,)):
                nonlocal ncopy
                for _ in range(n):
                    if not w_backlog:
                        return
                    tt, o5 = w_backlog.pop(0)
                    if o5 % 2 == 0:
                        w_ot[tt] = asb.tile([128, 1024], BF16, tag="ot",
                                            bufs=OTB, name="ot")
                    ot = w_ot[tt]
                    wacc = ps.tile([128, 512], F32, tag="wps",
                                   bufs=2, name="wacc")
                    for h in range(HPC):
                        nc.tensor.matmul(
                            wacc[:],
                            attnT[(h, tt // 4)][
                                :, (tt % 4) * 128:(tt % 4 + 1) * 128],
                            wo_t[h][o5],
                            start=(h == 0), stop=(h == HPC - 1))
                    eng = engs[ncopy % len(engs)]
                    if eng is None:
                        # NB: GPSIMD cannot read PSUM on real HW; ACT is
                        # the busier engine during attention, so DVE takes
                        # 2 of every 3 staging copies
                        eng = (nc.vector, nc.scalar, nc.vector)[ncopy % 3]
                    osl = slice((o5 % 2) * 512, (o5 % 2 + 1) * 512)
                    if eng is nc.scalar:
                        eng.copy(ot[:, osl], wacc[:])
                    else:
                        eng.tensor_copy(ot[:, osl], wacc[:])
                    ncopy += 1
                    if o5 % 2 == 1:
                        nc.sync.dma_start(
                            out[tt * 128:(tt + 1) * 128,
                                (o5 - 1) * 512:(o5 + 1) * 512], ot[:])
                        del w_ot[tt]

            # ---- attention unit emitter (one head of one q5 tile) -------
            def attn_unit(q5, h, short_tail=False):
                qsl = slice(q5 * 512, (q5 + 1) * 512)
                nk = nk_of(q5)
                nfull = nk - 4 if mode == "causal" else nk
                if True:
                    hsl = slice(h * 128, (h + 1) * 128)
                    aps = ps.tile([128, 512], F32, tag="aps", bufs=2,
                                  name="aps")
                    esum = asb.tile([128, 512], F32R, tag="esum", bufs=3,
                                    name="esum")
                    first_q = [True]

                    def esum_acc(src):
                        if first_q[0]:
                            nc.gpsimd.tensor_copy(esum[:], src)
                            first_q[0] = False
                        else:
                            nc.gpsimd.tensor_add(esum[:], esum[:], src)

                    # Full-width chunks in pairs; pair sums reduce again
                    # to quads on DVE before Pool accumulates into esum.
                    # PV matmuls are emitted one chunk LATE so the next
                    # chunk's scores fill the exp latency in the in-order
                    # PE queue.
                    pes = []
                    pending_pv = []

                    def flush_pv():
                        for f in pending_pv:
                            f()
                        pending_pv.clear()

                    for p in range(nfull // 2):
                        kc0 = 2 * p
                        sg = ps.tile([128, 1024], F32, tag="spsg",
                                     bufs=2, name="sg")
                        for i in range(2):
                            kc = kc0 + i
                            nc.tensor.matmul(
                                sg[:, i * 512:(i + 1) * 512],
                                kT[h][:, kc * 128:(kc + 1) * 128],
                                qT[h][:, qsl],
                                start=True, stop=True)
                        if mode == "general":
                            g = asb.tile([128, 1024], F32, tag="gm", bufs=2)
                            nc.sync.dma_start(
                                g[:],
                                maskT[kc0 * 128:(kc0 + 2) * 128,
                                      qsl].rearrange("(j p) c -> p j c",
                                                     p=128))
                            nc.vector.tensor_add(sg[:], sg[:], g[:])
                        eg = asb.tile([128, 1024], BF16, tag="eg", bufs=EGB,
                                      name="eg")
                        nc.scalar.activation(
                            eg[:], sg[:],
                            mybir.ActivationFunctionType.Exp, scale=SC)
                        pe = asb.tile([128, 512], BF16, tag="pe", bufs=4,
                                      name="pe")
                        nc.vector.tensor_add(pe[:], eg[:, 0:512],
                                             eg[:, 512:1024])
                        pes.append(pe)
                        if len(pes) == 2:
                            qd = asb.tile([128, 512], BF16, tag="qd",
                                          bufs=3, name="qd")
                            nc.vector.tensor_add(qd[:], pes[0][:],
                                                 pes[1][:])
                            esum_acc(qd[:])
                            pes = []
                        if len(pending_pv) >= 2:
                            pending_pv.pop(0)()

                        def pv_pair(eg=eg, kc0=kc0):
                            for i in range(2):
                                kc = kc0 + i
                                nc.tensor.matmul(
                                    aps[:],
                                    vsb[kc][:, hsl],
                                    eg[:, i * 512:(i + 1) * 512],
                                    start=(kc == 0), stop=(kc == nk - 1))
                        pending_pv.append(pv_pair)
                    if pes:  # leftover pair (dense/general nfull%4 != 0)
                        esum_acc(pes[0][:])
                        pes = []
                    # causal diagonal chunks, trimmed to valid columns; two
                    # diagonal chunks share each 2-bank score tile.  All 4
                    # scores are emitted first, then W filler covers the exp
                    # latency, then the (ready) PV matmuls.
                    if mode == "causal":
                        dgs = [ps.tile([128, 1024], F32, tag="spsg",
                                       bufs=2, name="dg") for _ in range(2)]
                        for r in range(4):
                            kc = nfull + r
                            coff = 128 * r
                            boff = (r % 2) * 512 + coff
                            nc.tensor.matmul(
                                dgs[r // 2][:, boff:(r % 2) * 512 + 512],
                                kT[h][:, kc * 128:(kc + 1) * 128],
                                qT[h][:, q5 * 512 + coff:(q5 + 1) * 512],
                                start=True, stop=True)
                        ers = []
                        for r in range(4):
                            coff = 128 * r
                            boff = (r % 2) * 512 + coff
                            er = asb.tile([128, 512], BF16, tag="e1",
                                          bufs=3, name="er")
                            if r > 0:
                                # zero cols below coff so the pair adds
                                # below read defined data
                                nc.vector.memzero(er[:, 0:coff])
                            nc.scalar.activation(
                                er[:, coff:512],
                                dgs[r // 2][:, boff:(r % 2) * 512 + 512],
                                mybir.ActivationFunctionType.Exp,
                                scale=SC)
                            nc.vector.tensor_mul(
                                er[:, coff:coff + 128],
                                er[:, coff:coff + 128], m01t[:])
                            ers.append(er)
                        w_step(2)
                        flush_pv()
                        w_step(2)
                        for r in range(4):
                            kc = nfull + r
                            coff = 128 * r
                            nc.tensor.matmul(
                                aps[:, coff:512],
                                vsb[kc][:, hsl],
                                ers[r][:, coff:512],
                                start=(kc == 0), stop=(kc == nk - 1),
                                skip_group_check=True)
                        d01 = asb.tile([128, 512], BF16, tag="pe", bufs=4)
                        if short_tail:
                            # split-column normalize for the final unit:
                            # cols [0:384] finalize off er0..er2 early, only
                            # [384:512] waits for er3 through narrow ops,
                            # so the last W groups start sooner
                            at = asb.tile([128, 512], BF16, tag="aT",
                                          bufs=8, name="aT")
                            attnT[(h, q5)] = at
                            dsum = asb.tile([128, 512], F32, tag="dsum",
                                            bufs=2, name="dsum")
                            nc.vector.tensor_add(d01[:], ers[0][:],
                                                 ers[1][:])
                            esum_acc(d01[:])
                            nc.gpsimd.tensor_add(esum[:, 256:512],
                                                 esum[:, 256:512],
                                                 ers[2][:, 256:512])
                            nc.gpsimd.partition_all_reduce(
                                dsum[:, 0:384], esum[:, 0:384], 128,
                                bass_isa.ReduceOp.add)
                            nc.vector.reciprocal(dsum[:, 0:384],
                                                 dsum[:, 0:384])
                            nc.vector.tensor_mul(at[:, 0:384],
                                                 aps[:, 0:384],
                                                 dsum[:, 0:384])
                            nc.gpsimd.tensor_add(esum[:, 384:512],
                                                 esum[:, 384:512],
                                                 ers[3][:, 384:512])
                            nc.gpsimd.partition_all_reduce(
                                dsum[:, 384:512], esum[:, 384:512], 128,
                                bass_isa.ReduceOp.add)
                            nc.vector.reciprocal(dsum[:, 384:512],
                                                 dsum[:, 384:512])
                            nc.vector.tensor_mul(at[:, 384:512],
                                                 aps[:, 384:512],
                                                 dsum[:, 384:512])
                            return
                        d23 = asb.tile([128, 512], BF16, tag="pe", bufs=4)
                        dq = asb.tile([128, 512], BF16, tag="qd", bufs=3)
                        nc.vector.tensor_add(d01[:], ers[0][:], ers[1][:])
                        nc.vector.tensor_add(d23[:], ers[2][:], ers[3][:])
                        nc.vector.tensor_add(dq[:], d01[:], d23[:])
                        esum_acc(dq[:])
                        dsum = asb.tile([128, 512], F32, tag="dsum", bufs=2,
                                        name="dsum")
                        nc.gpsimd.partition_all_reduce(
                            dsum[:], esum[:], 128, bass_isa.ReduceOp.add)
                        nc.vector.reciprocal(dsum[:], dsum[:])
                        attnT[(h, q5)] = asb.tile([128, 512], BF16,
                                                  tag="aT", bufs=8,
                                                  name="aT")
                        nc.vector.tensor_mul(attnT[(h, q5)][:], aps[:],
                                             dsum[:])
                    else:
                        flush_pv()
                        # normalize: 1/colsum broadcast to all partitions
                        dsum = asb.tile([128, 512], F32, tag="dsum", bufs=2,
                                        name="dsum")
                        nc.gpsimd.partition_all_reduce(
                            dsum[:], esum[:], 128, bass_isa.ReduceOp.add)
                        nc.vector.reciprocal(dsum[:], dsum[:])
                        attnT[(h, q5)] = asb.tile([128, 512], BF16,
                                                  tag="aT", bufs=8,
                                                  name="aT")
                        nc.vector.tensor_mul(attnT[(h, q5)][:], aps[:],
                                             dsum[:])
                        w_step(4)

            # ---- Phase VQK + interleaved attention, per t5 --------------
            with (
                tc.tile_pool(name="w_p", bufs=1) as wp,
                tc.tile_pool(name="xt_p", bufs=2) as xp,
                tc.tile_pool(name="vqk_sb", bufs=2) as sb,
            ):
                wv5 = wp.tile([128, DC * 512], BF16, tag="wv5", name="wv5")
                wq5 = wp.tile([128, DC * 512], BF16, tag="wq5", name="wq5")
                wk5 = wp.tile([128, DC * 512], BF16, tag="wk5", name="wk5")
                wv_t = [wv5[:, dc * 512:(dc + 1) * 512] for dc in range(DC)]
                wq_t = [wq5[:, dc * 512:(dc + 1) * 512] for dc in range(DC)]
                wk_t = [wk5[:, dc * 512:(dc + 1) * 512] for dc in range(DC)]

                def dma_w(w5, w, half):
                    """One DMA for 8 contraction chunks of a weight matrix."""
                    dc0 = half * 8
                    nc.sync.dma_start(
                        w5[:, dc0 * 512:(dc0 + 8) * 512].rearrange(
                            "p (j c) -> p j c", j=8),
                        w[dc0 * 128:(dc0 + 8) * 128, :].rearrange(
                            "(j p) c -> p j c", p=128))

                def load_x(t5, quarter):
                    """One DMA for 4 contraction chunks of x's t5 column tile."""
                    tsl = slice(t5 * 512, (t5 + 1) * 512)
                    dc0 = quarter * 4
                    x5 = x5s[t5]
                    nc.sync.dma_start(
                        x5[:, dc0 * 512:(dc0 + 4) * 512].rearrange(
                            "p (j c) -> p j c", j=4),
                        xT[dc0 * 128:(dc0 + 4) * 128, tsl].rearrange(
                            "(j p) c -> p j c", p=128))

                def new_x5(t5):
                    x5s[t5] = xp.tile([128, DC * 512], BF16, tag="x5",
                                      bufs=2, name="x5")

                x5s = {}
                new_x5(0)
                # startup-critical DMA order: tiny dc=0 transfers first so
                # the V matmuls of t5=0 start ~immediately, then batched
                # transfers interleaved so supply stays ahead of demand
                nc.sync.dma_start(x5s[0][:, 0:512], xT[0:128, 0:512])
                nc.sync.dma_start(wv5[:, 0:512], wv[0:128, :])
                nc.sync.dma_start(x5s[0][:, 512:1024], xT[128:256, 0:512])
                nc.sync.dma_start(wv5[:, 512:1024], wv[128:256, :])
                nc.sync.dma_start(
                    x5s[0][:, 1024:4 * 512].rearrange("p (j c) -> p j c",
                                                      j=2),
                    xT[256:4 * 128, 0:512].rearrange("(j p) c -> p j c",
                                                     p=128))
                nc.sync.dma_start(
                    wv5[:, 1024:8 * 512].rearrange("p (j c) -> p j c", j=6),
                    wv[256:8 * 128, :].rearrange("(j p) c -> p j c", p=128))
                load_x(0, 1)
                dma_w(wv5, wv, 1)
                load_x(0, 2)
                dma_w(wq5, wq, 0)
                load_x(0, 3)
                dma_w(wq5, wq, 1)
                nc.sync.dma_start(cs_t[:], cs[:])
                dma_w(wk5, wk, 0)
                dma_w(wk5, wk, 1)
                if mode == "causal":
                    nc.sync.dma_start(m01t[:], m01[:])

                for t5 in range(T5):
                    tsl = slice(t5 * 512, (t5 + 1) * 512)
                    x5 = x5s.pop(t5)
                    xt = [x5[:, dc * 512:(dc + 1) * 512] for dc in range(DC)]
                    if t5 + 1 < T5:
                        new_x5(t5 + 1)
                        for qt_ in range(4):
                            load_x(t5 + 1, qt_)
                    if t5 == 1:
                        nc.sync.dma_start(
                            wo5[:].rearrange("p (j c) -> p j c", j=16),
                            wo[:].rearrange("(j p) c -> p j c", p=128))
                    # V part: natural [seq, feat] layout; the 4 accs are
                    # column-halves of two 2-bank tiles (tags shared with
                    # the attention score groups)
                    vbig = [ps.tile([128, 1024], F32, tag="spsg", bufs=2,
                                    name="vbig") for _ in range(2)]
                    vaccs = [vbig[t // 2][:, (t % 2) * 512:(t % 2 + 1) * 512]
                             for t in range(4)]
                    for dc in range(DC):
                        for t in range(4):
                            nc.tensor.matmul(
                                vaccs[t][:],
                                xt[dc][:, t * 128:(t + 1) * 128],
                                wv_t[dc],
                                start=(dc == 0), stop=(dc == DC - 1))
                    for t in range(4):
                        nc.scalar.copy(vsb[t5 * 4 + t][:], vaccs[t][:])
                    # QK part: transposed [feat, seq] layout + RoPE.  The
                    # previous t5's attention units interleave between the
                    # dense QK matmul blocks: their latency chains resolve
                    # while PE streams QK work.
                    ct = cs_t[:, tsl]
                    st = cs_t[:, S + t5 * 512:S + (t5 + 1) * 512]
                    nqk = 0
                    for h in range(HPC):
                        hsl = slice(h * 128, (h + 1) * 128)
                        for w_t, dstT in ((wq_t, qT), (wk_t, kT)):
                            acc = ps.tile([128, 512], F32,
                                          tag=("wps", "aps")[nqk % 2],
                                          bufs=2, name="qkps")
                            for dc in range(DC):
                                nc.tensor.matmul(
                                    acc[:], w_t[dc][:, hsl], xt[dc],
                                    start=(dc == 0), stop=(dc == DC - 1))
                            # stage the acc halves out of PSUM with two
                            # fast copies so its bank frees quickly (the
                            # tag is shared with attention/W psum tiles);
                            # RoPE then runs in bf16 (2x DVE) off base-0
                            # tiles (SB+SB ops need equal base partitions).
                            # rows 0:64 = "a" (even), 64:128 = "b"
                            ca = sb.tile([64, 512], BF16, tag="ca",
                                         bufs=2, name="ca")
                            cb = sb.tile([64, 512], BF16, tag="cb",
                                         bufs=2, name="cb")
                            nc.scalar.copy(ca[:], acc[0:64, :])
                            nc.vector.tensor_copy(cb[:], acc[64:128, :])
                            a, b = ca[:], cb[:]
                            m1 = sb.tile([64, 512], BF16, tag="m1", bufs=2)
                            m2 = sb.tile([64, 512], BF16, tag="m2", bufs=2)
                            m3 = sb.tile([64, 512], BF16, tag="m3", bufs=2)
                            m4 = sb.tile([64, 512], BF16, tag="m4", bufs=2)
                            nc.vector.tensor_mul(m1[:], a, ct)
                            nc.vector.tensor_mul(m2[:], b, st)
                            nc.vector.tensor_mul(m3[:], a, st)
                            nc.vector.tensor_mul(m4[:], b, ct)
                            nc.gpsimd.tensor_sub(dstT[h][0:64, tsl],
                                                 m1[:], m2[:])
                            nc.gpsimd.tensor_add(dstT[h][64:128, tsl],
                                                 m3[:], m4[:])
                            if mode == "causal" and t5 >= 1 and nqk % 2 == 1:
                                attn_unit(t5 - 1, nqk // 2)
                                if nqk // 2 == HPC - 1:
                                    w_push(t5 - 1)
                            nqk += 1

                # final attention stages + W drain
                if mode == "causal":
                    for h in range(HPC):
                        attn_unit(T5 - 1, h, short_tail=(h == HPC - 1))
                    w_push(T5 - 1)
                else:
                    for q5 in range(T5):
                        for h in range(HPC):
                            attn_unit(q5, h)
                        w_push(q5)
                w_step(len(w_backlog), engs=(nc.vector, nc.scalar))

    nc.finalize()
    return nc


_PROGRAMS = {}


def _get_program(mode):
    if mode not in _PROGRAMS:
        _PROGRAMS[mode] = _build_program(mode)
    return _PROGRAMS[mode]


def _rope_perm():
    p = np.empty(HD, np.int64)
    p[: HD // 2] = np.arange(0, HD, 2)
    p[HD // 2:] = np.arange(1, HD, 2)
    return p


def _detect_mode(mask2):
    if not np.any(mask2):
        return "dense"
    iu = np.triu_indices(S, 1)
    il = np.tril_indices(S, 0)
    if not np.any(mask2[il]) and np.all(mask2[iu] <= -1.0e4):
        return "causal"
    return "general"


def _prepare_inputs(x, wq, wk, wv, wo, cos, sin, mask, start_p, seq_l):
    x = np.asarray(x, np.float32)
    wq = np.asarray(wq, np.float32)
    wk = np.asarray(wk, np.float32)
    wv = np.asarray(wv, np.float32)
    wo = np.asarray(wo, np.float32)
    cos = np.asarray(cos, np.float32)
    sin = np.asarray(sin, np.float32)
    mask2 = np.asarray(mask, np.float32).reshape(S, S)
    sp = int(np.asarray(start_p))
    sl = int(np.asarray(seq_l))
    assert sl == S, f"kernel hardcodes seq_l == {S}, got {sl}"

    mode = _detect_mode(mask2)

    cs = np.empty((64, 2 * S), np.float32)
    cs[:, 0:S] = cos[sp:sp + sl].T
    cs[:, S:2 * S] = sin[sp:sp + sl].T
    cs = cs.astype(BF)

    perm = _rope_perm()
    in_maps = []
    shared = {"cs": cs}
    if mode == "causal":
        i = np.arange(128)[:, None]
        j = np.arange(128)[None, :]
        shared["m01"] = (j >= i).astype(BF)
    if mode == "general":
        shared["maskT"] = np.ascontiguousarray(mask2.T * math.sqrt(HD))

    xTs = [np.ascontiguousarray(x[b].T).astype(BF) for b in range(B)]
    for core in range(NCORES):
        b = core // HGRP
        g = core % HGRP
        hs = g * HPC  # first global head of this core
        cols = []
        for h in range(HPC):
            base = (hs + h) * HD
            cols.append(base + perm)
        cols = np.concatenate(cols)
        csl = slice(hs * HD, hs * HD + FPC)
        in_maps.append({
            "xT": xTs[b],
            "wq": np.ascontiguousarray(wq[:, cols]).astype(BF),
            "wk": np.ascontiguousarray(wk[:, cols]).astype(BF),
            "wv": np.ascontiguousarray(wv[:, csl]).astype(BF),
            "wo": np.ascontiguousarray(wo[csl, :]).astype(BF),
            **shared,
        })
    return mode, in_maps


def run(inputs, trace=False):
    mode, in_maps = _prepare_inputs(**inputs)
    nc = _get_program(mode)
    res = run_bass_kernel_spmd(nc, in_maps, list(range(NCORES)), trace=trace)
    out = np.empty((B, S, D), np.float32)
    for b in range(B):
        acc = res.results[b * HGRP]["out"].astype(np.float32)
        for g in range(1, HGRP):
            acc = acc + res.results[b * HGRP + g]["out"].astype(np.float32)
        out[b] = acc
    return out, res


def kernel(**inputs):
    out, _ = run(inputs, trace=False)
    return out
